# revision 25
# baseline (speedup 1.0000x reference)
"""Trainium2 Bass kernel for CachedLightningIndexer-style scoring.

Reference computation (b=2, t=s=4096, d_model=2048, heads=2, dim=32):
    q = (x @ wq).reshape(b, t, 2, 32); k = x @ wk; w = x @ ww
    scores[b,t,s] = sum_h w[b,t,h] * relu(q[b,t,h,:] . k[b,s,:])

Sharding (8 cores): output grid of (t-quarter, s-quarter) blocks with
core c -> (tau[c], sigma[c]) chosen so the pairwise AllGather groups
[[0,1],[2,3],[4,5],[6,7]] deliver every core the keys of its sigma
quarter in slot 0 of the gathered buffer (fully uniform SPMD).

Each core streams ONLY its own t-slab of x (bf16, 8.4MB) plus one
combined weight matrix W = [wq(64) | wk(32)] (M=96). A single
projection pass (4 rounds of 512 cols, PSUM double-buffered) produces
q0|q1 and this quarter's keys. Keys bounce through DRAM for the
pairwise AllGather; gathered sigma-keys are DMA-duplicated into the
rot4 kT4 layout. The per-token mixing weights w0,w1 are computed on
the host (f32) and shipped pre-transposed as a tiny [128, NJ, 6]
tensor: (|w0|, |w1|, w1, sign(w0), s1clamp(w1), s2clamp(w1)).

Scores: per 128-row t-tile and 1024-col s-chunk, four K=32 matmuls
rotate over the four 32-row PE tile groups (concurrent execution),
leaving d_h = q_h . k in PSUM. Host pre-sorts t rows within each
quarter by the sign class of (w0, w1), so most tiles have uniform
signs and the epilogue is 3 ops spread over three engines:
    a0  = Relu(|w0| * d0)                      ACT  (magnitude)
    h1  = (w1 * d1) max/min 0                  DVE  (signed)
    out = s0 * a0 + h1                         Pool (scalar_tensor_tensor)
Mixed-sign leftover tiles clamp per-partition with +-BIG vectors.
Output tiles stream to DRAM as bf16; the host un-permutes rows.
"""

import numpy as np
import ml_dtypes

import concourse.bass as bass
import concourse.mybir as mybir
import concourse.tile as tile
from concourse import bacc
from concourse.bass_utils import run_bass_kernel_spmd

BF16 = ml_dtypes.bfloat16

D_MODEL = 2048
B = 2
T = 4096
DIM = 32
N_CORES = 8
Q = 2048              # t cols per core
S = 2048              # s cols per core
KT = D_MODEL // 128   # 16
NJ = Q // 128         # 16
RC = 1024             # projection round width
NR = Q // RC          # 2 rounds
SC = 1024             # scores chunk width
BIG = 1e30

TAU = [0, 1, 1, 0, 2, 3, 3, 2]
SIGMA = [0, 0, 1, 1, 2, 2, 3, 3]
GROUPS = [[0, 1], [2, 3], [4, 5], [6, 7]]

_cached = {}


def _build(tile_classes):
    """tile_classes: tuple of NJ entries; 0..3 = uniform sign class of
    (w0, w1) for that 128-row t-tile (bit1 = w0<0, bit0 = w1<0),
    -1 = mixed tile."""
    out_dt = mybir.dt.bfloat16
    f32 = mybir.dt.float32
    bf16 = mybir.dt.bfloat16
    Alu = mybir.AluOpType
    Act = mybir.ActivationFunctionType

    nc = bacc.Bacc("TRN2", target_bir_lowering=False, debug=False,
                   num_devices=N_CORES)
    xTt = nc.dram_tensor("xTt", [128, KT, Q], bf16, kind="ExternalInput").ap()
    wqwk = nc.dram_tensor("wqwk", [128, KT, 96], bf16,
                          kind="ExternalInput").ap()
    wext = nc.dram_tensor("wext", [128, NJ, 6], f32,
                          kind="ExternalInput").ap()
    out = nc.dram_tensor("out", [Q, S], out_dt, kind="ExternalOutput").ap()

    with tile.TileContext(nc) as tc:
        with tc.tile_pool(name="wpool", bufs=1) as wpool, \
             tc.tile_pool(name="xpool", bufs=1) as xpool, \
             tc.tile_pool(name="spool", bufs=1) as spool, \
             tc.tile_pool(name="dram", bufs=1, space="DRAM") as dram:

            # weights + keys path ride the gpsimd DMA queue, away from
            # the x stream
            wqwk_sb = wpool.tile([128, KT, 96], bf16)
            for i in range(4):
                nc.gpsimd.dma_start(wqwk_sb[:, i * 4:(i + 1) * 4, :],
                                    wqwk[:, i * 4:(i + 1) * 4, :])
            wv = wpool.tile([128, NJ, 6], f32)
            nc.gpsimd.dma_start(wv[:], wext[:])

            # x^T slab: [128, kt, 1024] pieces alternating between the
            # sync and gpsimd DMA queues for bandwidth (the scalar queue
            # is reserved for the keys-exchange path)
            xt = xpool.tile([128, KT, Q], bf16)
            xq = [nc.sync, nc.gpsimd]
            for r in range(NR):
                rsl = slice(r * RC, (r + 1) * RC)
                for kt in range(KT):
                    xq[kt % 2].dma_start(xt[:, kt, rsl], xTt[:, kt, rsl])

            # rot4 layouts: qT4 rows [q0, q1, q0, q1]; kT4 keys x4
            qT4 = spool.tile([128, Q], bf16)
            kT4 = spool.tile([128, S], bf16)
            k_sb = spool.tile([32, S], bf16)     # local keys staging

            # three exchange slices: 512 + 512 + 1024 cols (the first is
            # small so its AllGather dispatches as early as possible)
            KW = [512, 512, 1024]
            KO = [0, 512, 1024]
            kb_in = [dram.tile([32, KW[i]], bf16, tag=f"kbi{i}",
                               name=f"kbi{i}") for i in range(3)]
            kb_out = [dram.tile([64, KW[i]], bf16, tag=f"kbo{i}",
                                name=f"kbo{i}") for i in range(3)]

            warm = wpool.tile([128, 512], bf16)
            nc.vector.memset(warm[:], 0.25)

            with tc.tile_pool(name="pspool", bufs=2, space="PSUM") as pspool:
                # PE warm-up: trip the HAM clock gate before real work.
                # No DMA dependency, so it starts immediately.
                psw = pspool.tile([128, RC], f32, tag="ps")
                for _ in range(8):
                    nc.tensor.matmul(psw[0:32, 0:512], lhsT=warm[:, 0:32],
                                     rhs=warm[:, :])

                # single projection pass: q0|q1|keys, slices of 512/512/1024
                ps01 = pspool.tile([128, RC], f32, tag="ps")
                ps2 = pspool.tile([128, RC], f32, tag="ps")
                for i in range(3):
                    ssl = slice(KO[i], KO[i] + KW[i])
                    ps = ps01 if i < 2 else ps2
                    for kt in range(KT):
                        for n in range(KW[i] // 512):
                            psl = slice(KO[i] % RC + n * 512,
                                        KO[i] % RC + (n + 1) * 512)
                            gsl = slice(KO[i] + n * 512,
                                        KO[i] + (n + 1) * 512)
                            nc.tensor.matmul(ps[0:96, psl],
                                             lhsT=wqwk_sb[:, kt, :],
                                             rhs=xt[:, kt, gsl],
                                             start=(kt == 0),
                                             stop=(kt == KT - 1))
                    pssl = slice(KO[i] % RC, KO[i] % RC + KW[i])
                    # keys first: they gate the AllGather
                    nc.scalar.copy(k_sb[:, ssl], ps[64:96, pssl])
                    nc.scalar.copy(qT4[0:64, ssl], ps[0:64, pssl])
                    # keys exchange: DRAM bounce -> pairwise AllGather;
                    # kb rides the scalar DMA queue, CC on the Pool queue
                    nc.scalar.dma_start(kb_in[i][:], k_sb[:, ssl])
                    nc.gpsimd.collective_compute(
                        "AllGather", Alu.bypass, replica_groups=GROUPS,
                        ins=[kb_in[i].opt()], outs=[kb_out[i].opt()])
                    # rot4 dup of q (DVE, its queue is clean here)
                    nc.vector.tensor_copy(qT4[64:128, ssl], qT4[0:64, ssl])

                # gathered sigma-keys -> rot4 kT4, emitted after ALL kb
                # DMAs so the scalar ring never blocks a later kb behind
                # a kT4 load that waits on a collective
                for i in range(3):
                    ssl = slice(KO[i], KO[i] + KW[i])
                    for g in range(4):
                        nc.scalar.dma_start(kT4[32 * g:32 * (g + 1), ssl],
                                            kb_out[i][0:32, :])

            # ---- scores ---- (projection PSUM pool is closed, so all 8
            # banks are available: deeper d pipeline)
            with tc.tile_pool(name="dpool", bufs=4, space="PSUM") as dpool, \
                 tc.tile_pool(name="rpool", bufs=4) as rpool, \
                 tc.tile_pool(name="opool", bufs=4) as opool:
                    for c in range(2):
                        csl = slice(c * SC, (c + 1) * SC)
                        for jj in range(NJ):
                            tsl = slice(jj * 128, (jj + 1) * 128)
                            cls = tile_classes[jj]
                            idx = c * NJ + jj
                            w0a = wv[:, jj, 0:1]
                            w1a = wv[:, jj, 1:2]
                            w1s = wv[:, jj, 2:3]
                            sg0 = wv[:, jj, 3:4]
                            sv1 = wv[:, jj, 4:5]
                            sv2 = wv[:, jj, 5:6]
                            d0 = dpool.tile([128, SC], f32, tag="d")
                            d1 = dpool.tile([128, SC], f32, tag="d")
                            for n in range(2):
                                sl = slice(n * 512, (n + 1) * 512)
                                ksl = slice(c * SC + n * 512,
                                            c * SC + (n + 1) * 512)
                                for dd, gb in ((d0, 0), (d1, 1)):
                                    g = gb + 2 * n
                                    qrow = slice(32 * g, 32 * (g + 1))
                                    nc.tensor.matmul(dd[:, sl],
                                                     lhsT=qT4[qrow, tsl],
                                                     rhs=kT4[qrow, ksl],
                                                     tile_position=(32 * g, 0))
                            ot = opool.tile([128, SC], out_dt, tag="ot")
                            a0 = rpool.tile([128, SC], bf16, tag="h0")
                            h1 = rpool.tile([128, SC], bf16, tag="h1")
                            # a0 = |w0| * relu(d0)  (magnitude, ACT)
                            nc.scalar.activation(a0[:], d0[:], Act.Relu,
                                                 0.0, w0a)
                            if cls >= 0:
                                s0 = 1.0 if cls < 2 else -1.0
                                s1 = 1.0 if cls % 2 == 0 else -1.0
                                if cls < 3 and idx % 3 == 1:
                                    # rebalance: this tile's d1 evac on ACT
                                    # (magnitude); combine is a plain tt
                                    nc.scalar.activation(h1[:], d1[:],
                                                         Act.Relu, 0.0, w1a)
                                    args = ((a0, h1, Alu.add) if cls == 0 else
                                            (a0, h1, Alu.subtract) if cls == 1
                                            else (h1, a0, Alu.subtract))
                                else:
                                    # h1 = w1*relu(d1) signed (DVE)
                                    nc.vector.tensor_scalar(
                                        h1[:], d1[:], w1s, 0.0, Alu.mult,
                                        Alu.max if s1 > 0 else Alu.min)
                                    args = ((a0, h1, Alu.add) if s0 > 0 else
                                            (h1, a0, Alu.subtract))
                                # combine: Pool takes some chunk-1 tiles
                                # (its queue is blocked by collectives
                                # until ~the end of chunk 0); DVE the rest
                                eng = (nc.gpsimd
                                       if (c == 1 and idx % 2 == 0)
                                       else nc.vector)
                                eng.tensor_tensor(ot[:], args[0][:],
                                                  args[1][:], args[2])
                            else:
                                h1x = rpool.tile([128, SC], bf16, tag="h1x")
                                t0 = rpool.tile([128, SC], bf16, tag="t0")
                                nc.vector.tensor_scalar(h1x[:], d1[:], w1s,
                                                        sv1, Alu.mult, Alu.max)
                                nc.vector.tensor_scalar(h1[:], h1x[:], sv2,
                                                        None, Alu.min)
                                # t0 = sign(w0)*a0 on ACT (Copy with scale)
                                nc.scalar.activation(t0[:], a0[:], Act.Copy,
                                                     0.0, sg0)
                                nc.vector.tensor_tensor(ot[:], t0[:], h1[:],
                                                        Alu.add)
                            oq = nc.sync if idx % 2 == 0 else nc.scalar
                            oq.dma_start(out[tsl, csl], ot[:])
    nc.compile()
    return nc


def _host_classes(x_flat, ww):
    """Per-quarter t-permutation grouping rows by (sign w0, sign w1).

    Returns (tile_classes, perms, w01): tile_classes is the per-tile
    class layout shared by all quarters (NJ entries, -1 = mixed);
    perms[q] is the within-quarter column permutation."""
    w01 = x_flat.astype(np.float32) @ np.asarray(ww, np.float32)
    cls = (w01[:, 0] < 0).astype(np.int32) * 2 + (w01[:, 1] < 0)
    counts = np.zeros((4, 4), np.int64)
    for qd in range(4):
        cq = cls[qd * Q:(qd + 1) * Q]
        for c in range(4):
            counts[qd, c] = int((cq == c).sum())
    fc = [int(counts[:, c].min()) // 128 for c in range(4)]
    while sum(fc) > NJ:            # paranoia; cannot trigger (sum<=16)
        fc[int(np.argmax(fc))] -= 1
    tile_classes = []
    for c in range(4):
        tile_classes += [c] * fc[c]
    tile_classes += [-1] * (NJ - len(tile_classes))
    perms = []
    for qd in range(4):
        cq = cls[qd * Q:(qd + 1) * Q]
        buckets = [np.nonzero(cq == c)[0] for c in range(4)]
        head, tail = [], []
        for c in range(4):
            take = fc[c] * 128
            head.append(buckets[c][:take])
            tail.append(buckets[c][take:])
        perms.append(np.concatenate(head + tail))
    return tuple(tile_classes), perms, w01


def _get_nc(tile_classes):
    if tile_classes not in _cached:
        _cached[tile_classes] = _build(tile_classes)
    return _cached[tile_classes]


def run(x, wq, wk, ww, trace=False, **kw):
    x_flat = np.ascontiguousarray(
        np.asarray(x, np.float32).reshape(B * T, D_MODEL))
    tile_classes, perms, w01 = _host_classes(x_flat, ww)
    nc = _get_nc(tile_classes)

    xT = x_flat.T.astype(BF16)                       # [2048, 8192]
    wfull = np.concatenate([np.asarray(wq, np.float32),
                            np.asarray(wk, np.float32)],
                           axis=1).astype(BF16)      # [2048, 96]
    wqwk_h = np.ascontiguousarray(
        wfull.reshape(KT, 128, 96).transpose(1, 0, 2))

    # per-quarter wext: (|w0|, |w1|, w1, sign(w0), s1clamp(w1), s2clamp(w1))
    wexts = []
    for qd in range(4):
        wq01 = w01[qd * Q:(qd + 1) * Q][perms[qd]]   # [2048, 2] permuted
        e = np.empty((NJ, 128, 6), np.float32)
        wt = wq01.reshape(NJ, 128, 2)
        e[:, :, 0] = np.abs(wt[:, :, 0])
        e[:, :, 1] = np.abs(wt[:, :, 1])
        e[:, :, 2] = wt[:, :, 1]
        e[:, :, 3] = np.sign(wt[:, :, 0])
        e[:, :, 4] = np.where(wt[:, :, 1] >= 0, 0.0, -BIG)
        e[:, :, 5] = np.where(wt[:, :, 1] >= 0, BIG, 0.0)
        wexts.append(np.ascontiguousarray(e.transpose(1, 0, 2)))

    in_maps = []
    for core in range(N_CORES):
        tq = TAU[core]
        cols = tq * Q + perms[tq]
        slab = np.ascontiguousarray(
            xT[:, cols].reshape(KT, 128, Q).transpose(1, 0, 2))
        in_maps.append({"xTt": slab, "wqwk": wqwk_h, "wext": wexts[tq]})

    res = run_bass_kernel_spmd(nc, in_maps, list(range(N_CORES)),
                               trace=trace, **kw)

    outp = np.empty((B, T, T), dtype=np.float32)
    for core in range(N_CORES):
        tq, sq = TAU[core], SIGMA[core]
        b = tq // 2
        blk = res.results[core]["out"].astype(np.float32)
        trows = (tq % 2) * Q + perms[tq]
        # keys were projected from the sigma-quarter's permuted slab, so
        # the s axis of blk is in perm order too
        scols = (sq % 2) * S + perms[sq]
        outp[b, trows[:, None], scols[None, :]] = blk
    return outp, res


def kernel(x, wq, wk, ww):
    outp, _ = run(x, wq, wk, ww, trace=False)
    return outp


# revision 29
# speedup vs baseline: 1.1676x; 1.1676x over previous
"""Trainium2 Bass kernel for CachedLightningIndexer-style scoring.

Reference computation (b=2, t=s=4096, d_model=2048, heads=2, dim=32):
    q = (x @ wq).reshape(b, t, 2, 32); k = x @ wk; w = x @ ww
    scores[b,t,s] = sum_h w[b,t,h] * relu(q[b,t,h,:] . k[b,s,:])

Sharding (8 cores): output grid of (t-quarter, s-quarter) blocks with
core c -> (tau[c], sigma[c]) chosen so the pairwise AllGather groups
[[0,1],[2,3],[4,5],[6,7]] deliver every core the keys of its sigma
quarter in slot 0 of the gathered buffer (fully uniform SPMD).

Each core streams ONLY its own t-slab of x (bf16, 8.4MB) plus one
combined weight matrix W = [wq(64) | wk(32)] (M=96). A single
projection pass (4 rounds of 512 cols, PSUM double-buffered) produces
q0|q1 and this quarter's keys. Keys bounce through DRAM for the
pairwise AllGather; gathered sigma-keys are DMA-duplicated into the
rot4 kT4 layout. The per-token mixing weights w0,w1 are computed on
the host (f32) and shipped pre-transposed as a tiny [128, NJ, 6]
tensor: (|w0|, |w1|, w1, sign(w0), s1clamp(w1), s2clamp(w1)).

Scores: per 128-row t-tile and 1024-col s-chunk, four K=32 matmuls
rotate over the four 32-row PE tile groups (concurrent execution),
leaving d_h = q_h . k in PSUM. Host pre-sorts t rows within each
quarter by the sign class of (w0, w1), so most tiles have uniform
signs and the epilogue is 3 ops spread over three engines:
    a0  = Relu(|w0| * d0)                      ACT  (magnitude)
    h1  = (w1 * d1) max/min 0                  DVE  (signed)
    out = s0 * a0 + h1                         Pool (scalar_tensor_tensor)
Mixed-sign leftover tiles clamp per-partition with +-BIG vectors.
Output tiles stream to DRAM as bf16; the host un-permutes rows.
"""

import numpy as np
import ml_dtypes

import concourse.bass as bass
import concourse.mybir as mybir
import concourse.tile as tile
from concourse import bacc
from concourse.bass_utils import run_bass_kernel_spmd

BF16 = ml_dtypes.bfloat16

D_MODEL = 2048
B = 2
T = 4096
DIM = 32
N_CORES = 8
Q = 2048              # t cols per core
S = 2048              # s cols per core
KT = D_MODEL // 128   # 16
NJ = Q // 128         # 16
RC = 1024             # projection round width
NR = Q // RC          # 2 rounds
SC = 1024             # scores chunk width
BIG = 1e30

TAU = [0, 1, 1, 0, 2, 3, 3, 2]
SIGMA = [0, 0, 1, 1, 2, 2, 3, 3]
GROUPS = [[0, 1], [2, 3], [4, 5], [6, 7]]

_cached = {}


def _build(tile_classes):
    """tile_classes: tuple of NJ entries; 0..3 = uniform sign class of
    (w0, w1) for that 128-row t-tile (bit1 = w0<0, bit0 = w1<0),
    -1 = mixed tile."""
    out_dt = mybir.dt.bfloat16
    f32 = mybir.dt.float32
    bf16 = mybir.dt.bfloat16
    Alu = mybir.AluOpType
    Act = mybir.ActivationFunctionType

    nc = bacc.Bacc("TRN2", target_bir_lowering=False, debug=False,
                   num_devices=N_CORES)
    xTt = nc.dram_tensor("xTt", [128, KT, Q], bf16, kind="ExternalInput").ap()
    wqwk = nc.dram_tensor("wqwk", [128, KT, 96], bf16,
                          kind="ExternalInput").ap()
    wext = nc.dram_tensor("wext", [128, NJ, 6], f32,
                          kind="ExternalInput").ap()
    out = nc.dram_tensor("out", [Q, S], out_dt, kind="ExternalOutput").ap()

    with tile.TileContext(nc) as tc:
        with tc.tile_pool(name="wpool", bufs=1) as wpool, \
             tc.tile_pool(name="xpool", bufs=1) as xpool, \
             tc.tile_pool(name="spool", bufs=1) as spool, \
             tc.tile_pool(name="dram", bufs=1, space="DRAM") as dram:

            # weights + keys path ride the gpsimd DMA queue, away from
            # the x stream
            wqwk_sb = wpool.tile([128, KT, 96], bf16)
            for i in range(4):
                nc.gpsimd.dma_start(wqwk_sb[:, i * 4:(i + 1) * 4, :],
                                    wqwk[:, i * 4:(i + 1) * 4, :])
            wv = wpool.tile([128, NJ, 6], f32)
            nc.gpsimd.dma_start(wv[:], wext[:])

            # x^T slab: [128, kt, 1024] pieces alternating between the
            # sync and scalar DMA queues for bandwidth
            xt = xpool.tile([128, KT, Q], bf16)
            xq = [nc.sync, nc.scalar]
            for r in range(NR):
                rsl = slice(r * RC, (r + 1) * RC)
                for kt in range(KT):
                    xq[kt % 2].dma_start(xt[:, kt, rsl], xTt[:, kt, rsl])

            # rot4 layouts: qT4 rows [q0, q1, q0, q1]; kT4 keys x4
            qT4 = spool.tile([128, Q], bf16)
            kT4 = spool.tile([128, S], bf16)
            k_sb = spool.tile([32, S], bf16)     # local keys staging

            kb_in = [dram.tile([32, SC], bf16, tag=f"kbi{h}", name=f"kbi{h}")
                     for h in range(2)]
            kb_out = [dram.tile([64, SC], bf16, tag=f"kbo{h}", name=f"kbo{h}")
                      for h in range(2)]

            warm = wpool.tile([128, 512], bf16)
            nc.vector.memset(warm[:], 0.25)

            with tc.tile_pool(name="pspool", bufs=2, space="PSUM") as pspool:
                # PE warm-up: trip the HAM clock gate before real work.
                # No DMA dependency, so it starts immediately.
                psw = pspool.tile([128, RC], f32, tag="ps")
                for _ in range(8):
                    nc.tensor.matmul(psw[0:32, 0:512], lhsT=warm[:, 0:32],
                                     rhs=warm[:, :])

                # single projection pass: q0|q1|keys, 2 rounds of 1024
                for r in range(NR):
                    rsl = slice(r * RC, (r + 1) * RC)
                    ps = pspool.tile([128, RC], f32, tag="ps")
                    for kt in range(KT):
                        for n in range(2):
                            nsl = slice(n * 512, (n + 1) * 512)
                            gsl = slice(r * RC + n * 512,
                                        r * RC + (n + 1) * 512)
                            nc.tensor.matmul(ps[0:96, nsl],
                                             lhsT=wqwk_sb[:, kt, :],
                                             rhs=xt[:, kt, gsl],
                                             start=(kt == 0),
                                             stop=(kt == KT - 1))
                    # keys first: they gate the AllGather
                    nc.scalar.copy(k_sb[:, rsl], ps[64:96, :])
                    nc.scalar.copy(qT4[0:64, rsl], ps[0:64, :])

                    # keys exchange: DRAM bounce -> pairwise AllGather,
                    # all on the otherwise-idle gpsimd DMA queue
                    nc.gpsimd.dma_start(kb_in[r][:], k_sb[:, rsl])
                    nc.gpsimd.collective_compute(
                        "AllGather", Alu.bypass, replica_groups=GROUPS,
                        ins=[kb_in[r].opt()], outs=[kb_out[r].opt()])
                    for g in range(4):
                        nc.gpsimd.dma_start(kT4[32 * g:32 * (g + 1), rsl],
                                            kb_out[r][0:32, :])
                    # rot4 dup of q (Pool, idle during projection)
                    nc.gpsimd.tensor_copy(qT4[64:128, rsl], qT4[0:64, rsl])

            # ---- scores ---- (projection PSUM pool is closed, so all 8
            # banks are available: deeper d pipeline)
            with tc.tile_pool(name="dpool", bufs=4, space="PSUM") as dpool, \
                 tc.tile_pool(name="rpool", bufs=4) as rpool, \
                 tc.tile_pool(name="opool", bufs=4) as opool:
                    for c in range(2):
                        csl = slice(c * SC, (c + 1) * SC)
                        for jj in range(NJ):
                            tsl = slice(jj * 128, (jj + 1) * 128)
                            cls = tile_classes[jj]
                            idx = c * NJ + jj
                            w0a = wv[:, jj, 0:1]
                            w1a = wv[:, jj, 1:2]
                            w1s = wv[:, jj, 2:3]
                            sg0 = wv[:, jj, 3:4]
                            sv1 = wv[:, jj, 4:5]
                            sv2 = wv[:, jj, 5:6]
                            d0 = dpool.tile([128, SC], f32, tag="d")
                            d1 = dpool.tile([128, SC], f32, tag="d")
                            for n in range(2):
                                sl = slice(n * 512, (n + 1) * 512)
                                ksl = slice(c * SC + n * 512,
                                            c * SC + (n + 1) * 512)
                                for dd, gb in ((d0, 0), (d1, 1)):
                                    g = gb + 2 * n
                                    qrow = slice(32 * g, 32 * (g + 1))
                                    nc.tensor.matmul(dd[:, sl],
                                                     lhsT=qT4[qrow, tsl],
                                                     rhs=kT4[qrow, ksl],
                                                     tile_position=(32 * g, 0))
                            ot = opool.tile([128, SC], out_dt, tag="ot")
                            a0 = rpool.tile([128, SC], bf16, tag="h0")
                            h1 = rpool.tile([128, SC], bf16, tag="h1")
                            # a0 = |w0| * relu(d0)  (magnitude, ACT)
                            nc.scalar.activation(a0[:], d0[:], Act.Relu,
                                                 0.0, w0a)
                            if cls >= 0:
                                s0 = 1.0 if cls < 2 else -1.0
                                s1 = 1.0 if cls % 2 == 0 else -1.0
                                if cls < 3 and idx % 3 == 1:
                                    # rebalance: this tile's d1 evac on ACT
                                    # (magnitude); combine is a plain tt
                                    nc.scalar.activation(h1[:], d1[:],
                                                         Act.Relu, 0.0, w1a)
                                    args = ((a0, h1, Alu.add) if cls == 0 else
                                            (a0, h1, Alu.subtract) if cls == 1
                                            else (h1, a0, Alu.subtract))
                                else:
                                    # h1 = w1*relu(d1) signed (DVE)
                                    nc.vector.tensor_scalar(
                                        h1[:], d1[:], w1s, 0.0, Alu.mult,
                                        Alu.max if s1 > 0 else Alu.min)
                                    args = ((a0, h1, Alu.add) if s0 > 0 else
                                            (h1, a0, Alu.subtract))
                                # combine: Pool takes every 3rd (it is slow
                                # but otherwise idle); DVE the rest
                                eng = nc.gpsimd if idx % 3 == 2 else nc.vector
                                eng.tensor_tensor(ot[:], args[0][:],
                                                  args[1][:], args[2])
                            else:
                                h1x = rpool.tile([128, SC], bf16, tag="h1x")
                                t0 = rpool.tile([128, SC], bf16, tag="t0")
                                nc.vector.tensor_scalar(h1x[:], d1[:], w1s,
                                                        sv1, Alu.mult, Alu.max)
                                nc.vector.tensor_scalar(h1[:], h1x[:], sv2,
                                                        None, Alu.min)
                                # t0 = sign(w0)*a0 on ACT (Copy with scale)
                                nc.scalar.activation(t0[:], a0[:], Act.Copy,
                                                     0.0, sg0)
                                nc.vector.tensor_tensor(ot[:], t0[:], h1[:],
                                                        Alu.add)
                            oq = nc.sync if idx % 2 == 0 else nc.scalar
                            oq.dma_start(out[tsl, csl], ot[:])
    nc.compile()
    return nc


def _host_classes(x_flat, ww):
    """Per-quarter t-permutation grouping rows by (sign w0, sign w1).

    Returns (tile_classes, perms, w01): tile_classes is the per-tile
    class layout shared by all quarters (NJ entries, -1 = mixed);
    perms[q] is the within-quarter column permutation."""
    w01 = x_flat.astype(np.float32) @ np.asarray(ww, np.float32)
    cls = (w01[:, 0] < 0).astype(np.int32) * 2 + (w01[:, 1] < 0)
    counts = np.zeros((4, 4), np.int64)
    for qd in range(4):
        cq = cls[qd * Q:(qd + 1) * Q]
        for c in range(4):
            counts[qd, c] = int((cq == c).sum())
    fc = [int(counts[:, c].min()) // 128 for c in range(4)]
    while sum(fc) > NJ:            # paranoia; cannot trigger (sum<=16)
        fc[int(np.argmax(fc))] -= 1
    tile_classes = []
    for c in range(4):
        tile_classes += [c] * fc[c]
    tile_classes += [-1] * (NJ - len(tile_classes))
    perms = []
    for qd in range(4):
        cq = cls[qd * Q:(qd + 1) * Q]
        buckets = [np.nonzero(cq == c)[0] for c in range(4)]
        head, tail = [], []
        for c in range(4):
            take = fc[c] * 128
            head.append(buckets[c][:take])
            tail.append(buckets[c][take:])
        perms.append(np.concatenate(head + tail))
    return tuple(tile_classes), perms, w01


def _get_nc(tile_classes):
    if tile_classes not in _cached:
        _cached[tile_classes] = _build(tile_classes)
    return _cached[tile_classes]


def run(x, wq, wk, ww, trace=False, **kw):
    x_flat = np.ascontiguousarray(
        np.asarray(x, np.float32).reshape(B * T, D_MODEL))
    tile_classes, perms, w01 = _host_classes(x_flat, ww)
    nc = _get_nc(tile_classes)

    xT = x_flat.T.astype(BF16)                       # [2048, 8192]
    wfull = np.concatenate([np.asarray(wq, np.float32),
                            np.asarray(wk, np.float32)],
                           axis=1).astype(BF16)      # [2048, 96]
    wqwk_h = np.ascontiguousarray(
        wfull.reshape(KT, 128, 96).transpose(1, 0, 2))

    # per-quarter wext: (|w0|, |w1|, w1, sign(w0), s1clamp(w1), s2clamp(w1))
    wexts = []
    for qd in range(4):
        wq01 = w01[qd * Q:(qd + 1) * Q][perms[qd]]   # [2048, 2] permuted
        e = np.empty((NJ, 128, 6), np.float32)
        wt = wq01.reshape(NJ, 128, 2)
        e[:, :, 0] = np.abs(wt[:, :, 0])
        e[:, :, 1] = np.abs(wt[:, :, 1])
        e[:, :, 2] = wt[:, :, 1]
        e[:, :, 3] = np.sign(wt[:, :, 0])
        e[:, :, 4] = np.where(wt[:, :, 1] >= 0, 0.0, -BIG)
        e[:, :, 5] = np.where(wt[:, :, 1] >= 0, BIG, 0.0)
        wexts.append(np.ascontiguousarray(e.transpose(1, 0, 2)))

    in_maps = []
    for core in range(N_CORES):
        tq = TAU[core]
        cols = tq * Q + perms[tq]
        slab = np.ascontiguousarray(
            xT[:, cols].reshape(KT, 128, Q).transpose(1, 0, 2))
        in_maps.append({"xTt": slab, "wqwk": wqwk_h, "wext": wexts[tq]})

    res = run_bass_kernel_spmd(nc, in_maps, list(range(N_CORES)),
                               trace=trace, **kw)

    outp = np.empty((B, T, T), dtype=np.float32)
    for core in range(N_CORES):
        tq, sq = TAU[core], SIGMA[core]
        b = tq // 2
        blk = res.results[core]["out"].astype(np.float32)
        trows = (tq % 2) * Q + perms[tq]
        # keys were projected from the sigma-quarter's permuted slab, so
        # the s axis of blk is in perm order too
        scols = (sq % 2) * S + perms[sq]
        outp[b, trows[:, None], scols[None, :]] = blk
    return outp, res


def kernel(x, wq, wk, ww):
    outp, _ = run(x, wq, wk, ww, trace=False)
    return outp
